# revision 1
# baseline (speedup 1.0000x reference)
"""Trainium2 Bass kernel for nn_DistanceLoss (pairwise SmoothL1 distance loss).

reference:
    t[i,j] = sum_d smoothl1(x[i,d] - x[j,d])   (beta=1)  for x in {teacher, student}
    loss = sum |t/mean(t) - s/mean(s)|

identity used on device (per pair, with d = x_i - x_j):
    smoothl1(d) = 0.5 d^2 - 0.5 relu(|d|-1)^2
    sum_d 0.5 d^2 = 0.5 n_i + 0.5 n_j - G_ij       (Gram decomposition)

The pair matrix is symmetric, so only the upper triangle (i >= j) is computed.
Core k owns rows j == k (mod 8): local jl -> global j = 8*jl + k, and row jl
covers i in [8*jl, 512) (a core-independent range, so one program serves all
8 cores; the <=7 extra columns below the diagonal are ignored on the host).

Layout is transposed (d on partitions, i on the free dim). All four terms of
the identity accumulate into one PSUM tile [64, 512] per tensor via matmuls:
  0.5 n_i : stationary = 0.5-const       [128,64], moving = x^2 tile  [128,FD]
  0.5 n_j : stationary = 0.5*xj^2 slice  [128,64], moving = ones      [128,FD]
  -G_ij   : stationary = -xj slice       [128,64], moving = x tile    [128,FD]
  -0.5 c2 : stationary = -0.5*indicator  [128,64], moving = c2 tile   [128,FD]
where c2 = relu(|x_i - x_j| - 1)^2 comes from a fused custom DVE op; the
largest-FD j's instead use the Scalar engine's Abs (with per-partition bias)
plus stock vector ops, to keep both engines busy.
Host does the final (cheap) mean-normalize + abs-diff reduction in float64.
"""

import sys

for _p in ("/opt/trn_rl_repo", "/root/.axon_site/_ro/trn_rl_repo"):
    if _p not in sys.path:
        sys.path.insert(0, _p)

import numpy as np
import ml_dtypes

N = 512
D = 512
NCORES = 8
JB = N // NCORES  # 64 rows of the pair matrix per core
NT = D // 128  # 4 partition tiles of the transposed layout

import os
# jl < K1: ACT does Abs+Square (A2 path); K1 <= jl < K2: ACT does Abs (A1 path);
# else: custom DVE op, layout B (pair-sum accumulate) or layout T, whichever is
# cheaper for that jl's free dim.
K1 = int(os.environ.get("SL1_K1", "13"))
K2 = int(os.environ.get("SL1_K2", "15"))
GPV = os.environ.get("SL1_GPV", "12")  # "2": A2 v-step on gpsimd; "12": A1+A2
NOB = os.environ.get("SL1_NOB", "") == "1"

_CACHE = {}


def _fd(jl):
    return N - 8 * jl


def _register_custom_ops():
    from operator import add as _add

    import concourse.dve_ops as dve_ops
    from concourse.dve_spec import Spec, Src0, Src1, C0, C1, Zero, maxx, sq, lower
    from concourse.dve_uop import DveOpSpec

    def _reg(name, spec, rd1):
        for op in dve_ops.OPS:
            if op.name == name:
                return op
        row = dve_ops._CUSTOM_DVE_ROW_BASE + len(dve_ops.OPS)
        shas = {}
        for ver in ("v3", "v4"):
            s = DveOpSpec(name=name, opcode=row, uops=lower(spec, ver=ver),
                          rd1_en=rd1)
            shas[ver] = s.sha(ver)
        op = dve_ops.DveOp(name, spec, subdim=False, uops_sha=shas)
        dve_ops.OPS.append(op)
        dve_ops._SUB_OPCODE_FOR_NAME[name] = row
        dve_ops.CUSTOM_DVE_SPECS[name] = spec
        return op

    # layout T: out = relu(max(x - c0, c1 - x))^2 with c0 = xj+1, c1 = xj-1
    sl1c = _reg(
        "SL1C_ANT",
        Spec(
            body=sq(maxx(maxx(Src0 - C0, C1 - Src0), Zero)),
            reference=lambda in0, in1, s0, s1, imm2: np.square(
                np.maximum(np.maximum(in0 - s0, s1 - in0), 0.0)
            ).astype(np.float32),
        ),
        rd1=False,
    )

    # layout B: d = in0 - in1 (in1 = broadcast xj row), out = relu(|d|-1)^2,
    # accum_out = row-sum of out (the per-pair correction sum over d)
    from concourse.dve_spec import One

    _d = Src0 - Src1

    def _bref(in0, in1, s0, s1, imm2):
        d = in0.astype(np.float32) - in1
        b = np.square(np.maximum(np.abs(d) - 1.0, 0.0)).astype(np.float32)
        return b, b.reshape(b.shape[0], -1).sum(axis=-1, keepdims=True)

    sl1b = _reg(
        "SL1B_ANT",
        Spec(
            body=sq(maxx(maxx(_d, Zero - _d) - One, Zero)),
            accum=_add,
            reference=_bref,
        ),
        rd1=True,
    )
    return sl1c, sl1b


def _path(jl):
    if jl < K1:
        return "A2"
    if jl < K2:
        return "A1"
    if NOB:
        return "T"
    _bt = os.environ.get("SL1_BT", "t")
    if _bt == "b":
        return "B"
    if _bt == "tailb":
        fd = _fd(jl)
        return "B" if (fd <= 128 and 663 < 4 * (fd + 151)) else "T"
    if _bt == "tailp":
        fd = _fd(jl)
        return "P" if (fd <= 128 and 663 < 4 * (fd + 151)) else "T"
    if _bt == "t":
        return "T"
    fd = _fd(jl)
    b_cost = -(-fd // 128) * 663
    t_cost = 4 * (fd + 151)
    return "B" if b_cost <= t_cost else "T"


def _build_nc(repeat=1):
    import concourse.bacc as bacc
    import concourse.tile as tile
    from concourse import mybir

    sl1c, sl1b = _register_custom_ops()

    dt = mybir.dt
    nc = bacc.Bacc("TRN2", target_bir_lowering=False, debug=False,
                   num_devices=NCORES)

    dram = {}
    dram["m05i"] = nc.dram_tensor("m05i", [128, 128], dt.bfloat16,
                                  kind="ExternalInput").ap()
    for pfx in ("t", "s"):
        dram[pfx + "_xt"] = nc.dram_tensor(pfx + "_xt", [D, N], dt.bfloat16,
                                           kind="ExternalInput").ap()
        dram[pfx + "_xr"] = nc.dram_tensor(pfx + "_xr", [N, D], dt.bfloat16,
                                           kind="ExternalInput").ap()
        dram[pfx + "_xjr"] = nc.dram_tensor(pfx + "_xjr", [JB, D], dt.bfloat16,
                                            kind="ExternalInput").ap()
        dram[pfx + "_xj"] = nc.dram_tensor(pfx + "_xj", [D, JB], dt.bfloat16,
                                           kind="ExternalInput").ap()
        dram[pfx + "_jp1"] = nc.dram_tensor(pfx + "_jp1", [D, JB], dt.float32,
                                            kind="ExternalInput").ap()
        dram[pfx + "_jm1"] = nc.dram_tensor(pfx + "_jm1", [D, JB], dt.float32,
                                            kind="ExternalInput").ap()
        dram[pfx + "_out"] = nc.dram_tensor(pfx + "_out", [JB, N], dt.float32,
                                            kind="ExternalOutput").ap()
        dram[pfx + "_tc"] = nc.dram_tensor(pfx + "_tc", [128, 16], dt.float32,
                                           kind="ExternalOutput").ap()

    with tile.TileContext(nc) as tc:
        import contextlib

        with contextlib.ExitStack() as ctx:
            singles = ctx.enter_context(tc.tile_pool(name="singles", bufs=1))
            qpool = ctx.enter_context(tc.tile_pool(name="qpool", bufs=10))
            apool = ctx.enter_context(tc.tile_pool(name="apool", bufs=6))
            vpool = ctx.enter_context(tc.tile_pool(name="vpool", bufs=6))
            opool = ctx.enter_context(tc.tile_pool(name="opool", bufs=2))
            psp = ctx.enter_context(tc.tile_pool(name="psp", bufs=2, space="PSUM"))
            bcpool = ctx.enter_context(tc.tile_pool(name="bcpool", bufs=6))

            # shared constants
            zo = singles.tile([128, 128], dt.bfloat16)  # sliding -0.5 indicator
            nc.gpsimd.memset(zo, 0.0)
            nc.gpsimd.memset(zo[:, 63:64], -0.5)
            half32 = singles.tile([128, JB], dt.float32)
            nc.gpsimd.memset(half32, 0.5)
            ones32 = singles.tile([128, N], dt.float32)
            nc.gpsimd.memset(ones32, 1.0)
            m05i = singles.tile([128, 128], dt.bfloat16)  # -0.5 * identity
            nc.sync.dma_start(out=m05i, in_=dram["m05i"])

            _ord = ("s", "t") if os.environ.get("SL1_SWAP", "") == "1" else ("t", "s")
            _phases = [p for _ in range(repeat) for p in _ord]
            for _pi, pfx in enumerate(_phases):
                if _pi > 0 and os.environ.get("SL1_BAR", "0") == "1":
                    tc.strict_bb_all_engine_barrier()
                xt_sb = []
                xj_sb = []
                jp1 = []
                jm1 = []
                xr_sb = []
                xr_dma = []
                ctile = []
                for t in range(NT):
                    x = singles.tile([128, N], dt.bfloat16, tag=f"{pfx}_xt{t}")
                    nc.sync.dma_start(out=x, in_=dram[pfx + "_xt"][128 * t:128 * (t + 1), :])
                    xt_sb.append(x)
                    xj = singles.tile([128, JB], dt.bfloat16, tag=f"{pfx}_xj{t}")
                    nc.sync.dma_start(out=xj, in_=dram[pfx + "_xj"][128 * t:128 * (t + 1), :])
                    xj_sb.append(xj)
                    p1 = singles.tile([128, JB], dt.float32, tag=f"{pfx}_jp1{t}")
                    nc.sync.dma_start(out=p1, in_=dram[pfx + "_jp1"][128 * t:128 * (t + 1), :])
                    jp1.append(p1)
                    m1 = singles.tile([128, JB], dt.float32, tag=f"{pfx}_jm1{t}")
                    nc.sync.dma_start(out=m1, in_=dram[pfx + "_jm1"][128 * t:128 * (t + 1), :])
                    jm1.append(m1)
                    if any(_path(j) == "B" for j in range(JB)):
                        xr = singles.tile([128, D], dt.bfloat16, tag=f"{pfx}_xr{t}")
                        _xrd = nc.sync.dma_start(out=xr, in_=dram[pfx + "_xr"][128 * t:128 * (t + 1), :])
                        xr_sb.append(xr)
                        xr_dma.append(_xrd)
                        ct = singles.tile([128, JB], dt.float32, tag=f"{pfx}_ct{t}")
                        nc.gpsimd.memset(ct, 0.0)
                        ctile.append(ct)

                # derived per-tensor tiles
                negxj = []    # bf16, stationary for -G
                negxj32 = []  # fp32, ACT bias (= -xj)
                sq32 = []     # fp32 x^2 tiles, moving for n_i
                hsq32 = []    # fp32 0.5*xj^2 slices, stationary for n_j
                for t in range(NT):
                    nb = singles.tile([128, JB], dt.bfloat16, tag=f"{pfx}_negxj{t}")
                    nc.gpsimd.tensor_scalar(nb, xj_sb[t], -1.0, None, mybir.AluOpType.mult)
                    negxj.append(nb)
                    n32 = singles.tile([128, JB], dt.float32, tag=f"{pfx}_negxj32{t}")
                    # jp1 = xj + 1 (fp32 of the bf16-rounded xj) -> -(jp1 - 1) = -xj
                    nc.gpsimd.tensor_scalar(n32, jp1[t], 1.0, -1.0,
                                            mybir.AluOpType.subtract, mybir.AluOpType.mult)
                    negxj32.append(n32)
                    s32 = singles.tile([128, N], dt.float32, tag=f"{pfx}_sq{t}")
                    _steng = nc.gpsimd if os.environ.get("SL1_GPSETUP", "") == "1" else nc.vector
                    _steng.tensor_tensor(s32, xt_sb[t], xt_sb[t], mybir.AluOpType.mult)
                    sq32.append(s32)
                    h32 = singles.tile([128, JB], dt.float32, tag=f"{pfx}_hsq{t}")
                    nc.gpsimd.tensor_tensor(h32, xj_sb[t], xj_sb[t], mybir.AluOpType.mult)
                    nc.gpsimd.tensor_scalar(h32, h32, 0.5, None, mybir.AluOpType.mult)
                    hsq32.append(h32)

                import concourse.bass as bass
                b_jls = [j for j in range(JB) if _path(j) in ("B", "P")]
                b_slot = {j: i for i, j in enumerate(b_jls)}
                bc_all = None
                if b_jls:
                    bc_all = bcpool.tile([128, len(b_jls), D], dt.bfloat16,
                                         tag="bc_all", bufs=2)
                bc_dma = {}
                for jl in b_jls:
                    row = dram[pfx + "_xjr"][jl:jl + 1, :]
                    bcast_src = bass.AP(tensor=row.tensor, offset=row.offset,
                                        ap=[[0, 128]] + [list(p) for p in row.ap[1:]])
                    bc_dma[jl] = nc.sync.dma_start(out=bc_all[:, b_slot[jl], :],
                                                   in_=bcast_src)

                tc_sb = None
                xt3_rows = None
                if any(_path(j) == "P" for j in range(JB)):
                    xt3_rows = singles.tile([128, D], dt.bfloat16, tag=f"{pfx}_xr3")
                    nc.sync.dma_start(out=xt3_rows,
                                      in_=dram[pfx + "_xr"][384:512, :])
                    tc_sb = opool.tile([128, 16], dt.float32, tag="tc")
                    nc.gpsimd.memset(tc_sb, 0.0)

                acc = psp.tile([JB, N], dt.float32, tag=f"{pfx}_acc")

                # n_i, n_j, -G assembly matmuls (full width; sub-diagonal noise
                # is ignored by the host)
                first = True
                for t in range(NT):
                    nc.tensor.matmul(acc, half32, sq32[t], start=first, stop=False)
                    first = False
                for t in range(NT):
                    nc.tensor.matmul(acc, hsq32[t], ones32, start=False, stop=False)
                for t in range(NT):
                    nc.tensor.matmul(acc, negxj[t], xt_sb[t], start=False, stop=False)

                # per-j correction: c2 = relu(|x_i - x_j| - 1)^2 over i >= 8*jl.
                # A/T paths (layout T) feed -0.5-indicator matmuls into row jl;
                # B path (layout B) accumulates pair sums into ctile columns.
                # emit ACT-path and DVE-path j's interleaved so all engines
                # have runnable work from the start
                _a_js = [j for j in range(JB) if _path(j) in ("A1", "A2")]
                _d_js = [j for j in range(JB) if _path(j) in ("B", "T", "P")]
                _order = []
                _na, _nd = len(_a_js), len(_d_js)
                _ia = _id = 0
                _runway = int(os.environ.get("SL1_RUN", "1"))
                _runway = min(_runway, _nd)
                for _ in range(_runway):
                    _order.append(_d_js[_id]); _id += 1
                for _i in range(JB - _runway):
                    if _ia * (_nd - _runway) <= (_id - _runway) * _na and _ia < _na:
                        _order.append(_a_js[_ia]); _ia += 1
                    elif _id < _nd:
                        _order.append(_d_js[_id]); _id += 1
                    else:
                        _order.append(_a_js[_ia]); _ia += 1
                for jl in _order:
                    fd = _fd(jl)
                    i0 = N - fd
                    path = _path(jl)
                    if path == "P":
                        bc = bc_all[:, b_slot[jl], :]
                        junk = qpool.tile([128, D], dt.bfloat16, tag="junk")
                        _bop = nc.vector._custom_dve(
                            sl1b,
                            out=junk,
                            in0=xt3_rows,
                            in1=bc,
                            accum_out=tc_sb[:, jl - 48:jl - 47])
                        continue
                    if path == "B":
                        bc = bc_all[:, b_slot[jl], :]
                        _bcd = bc_dma[jl]
                        junk = qpool.tile([128, D], dt.bfloat16, tag="junk")
                        tb0 = (8 * jl) // 128
                        from concourse.tile_rust import add_dep_helper as _adh
                        for tb in range(tb0, NT):
                            p0 = 0
                            colt = vpool.tile([128, 1], dt.float32, tag="colt",
                                              bufs=8)
                            _bop = nc.vector._custom_dve(
                                sl1b,
                                out=junk[p0:128, :],
                                in0=xr_sb[tb][p0:128, :],
                                in1=bc[p0:128, :],
                                accum_out=colt[p0:128, 0:1])
                            _adh(_bop.ins, xr_dma[tb].ins,
                                 reason="custom-dve reads xr tile")
                            _adh(_bop.ins, _bcd.ins,
                                 reason="custom-dve reads bc tile")
                            nc.vector.tensor_copy(ctile[tb][p0:128, jl:jl + 1],
                                                  colt[p0:128, 0:1])
                        continue
                    if path == "A2":
                        a4 = apool.tile([128, NT, N], dt.bfloat16, tag="a4")
                        for t in range(NT):
                            nc.scalar.activation(a4[:, t, 0:fd], xt_sb[t][:, i0:N],
                                                 mybir.ActivationFunctionType.Abs,
                                                 bias=negxj32[t][:, jl:jl + 1],
                                                 scale=1.0)
                        v4 = vpool.tile([128, NT, N], dt.bfloat16, tag="v4")
                        veng = nc.gpsimd if "2" in GPV else nc.vector
                        veng.tensor_scalar(v4[:, :, 0:fd], a4[:, :, 0:fd],
                                           1.0, 0.0, mybir.AluOpType.subtract,
                                           mybir.AluOpType.max)
                        q4 = qpool.tile([128, NT, N], dt.bfloat16, tag="q4")
                        nc.scalar.activation(q4[:, :, 0:fd], v4[:, :, 0:fd],
                                             mybir.ActivationFunctionType.Square,
                                             bias=0.0, scale=1.0)
                    elif path == "A1":
                        nta = NT - int(os.environ.get("SL1_SPLIT", "1")) \
                            if jl == K2 - 2 else NT
                        a4 = apool.tile([128, NT, N], dt.bfloat16, tag="a4")
                        for t in range(nta):
                            nc.scalar.activation(a4[:, t, 0:fd], xt_sb[t][:, i0:N],
                                                 mybir.ActivationFunctionType.Abs,
                                                 bias=negxj32[t][:, jl:jl + 1],
                                                 scale=1.0)
                        v4 = vpool.tile([128, NT, N], dt.bfloat16, tag="v4")
                        veng = nc.gpsimd if "1" in GPV else nc.vector
                        veng.tensor_scalar(v4[:, 0:nta, 0:fd], a4[:, 0:nta, 0:fd],
                                           1.0, 0.0, mybir.AluOpType.subtract,
                                           mybir.AluOpType.max)
                        q4 = qpool.tile([128, NT, N], dt.bfloat16, tag="q4")
                        _sqeng = nc.gpsimd if os.environ.get("SL1_GPSQ", "") == "1" else nc.vector
                        _sqeng.tensor_tensor(q4[:, 0:nta, 0:fd], v4[:, 0:nta, 0:fd],
                                             v4[:, 0:nta, 0:fd], mybir.AluOpType.mult)
                        for t in range(nta, NT):
                            nc.vector._custom_dve(sl1c, out=q4[:, t, 0:fd],
                                                  in0=xt_sb[t][:, i0:N],
                                                  s0=jp1[t][:, jl:jl + 1],
                                                  s1=jm1[t][:, jl:jl + 1])
                    else:  # "T"
                        q4 = qpool.tile([128, NT, N], dt.bfloat16, tag="q4")
                        for t in range(NT):
                            nc.vector._custom_dve(sl1c, out=q4[:, t, 0:fd],
                                                  in0=xt_sb[t][:, i0:N],
                                                  s0=jp1[t][:, jl:jl + 1],
                                                  s1=jm1[t][:, jl:jl + 1])
                    for t in range(NT):
                        nc.tensor.matmul(acc[:, i0:N], zo[:, 63 - jl:127 - jl],
                                         q4[:, t, 0:fd],
                                         start=False, stop=False)

                # fold the layout-B correction columns into acc (transposed):
                # acc[jl, i] += -0.5 * ctile[b][i, jl]
                if any(_path(j) == "B" for j in range(JB)):
                    for b in range(NT):
                        ctb = bcpool.tile([128, JB], dt.bfloat16, tag="ctb")
                        nc.vector.tensor_copy(ctb, ctile[b])
                        nc.tensor.matmul(acc[:, 128 * b:128 * (b + 1)], ctb, m05i,
                                         start=False, stop=(b == NT - 1))
                else:
                    nc.tensor.matmul(acc[:, 0:128], zo[:, 64:128], m05i,
                                     start=False, stop=True)

                out_sb = opool.tile([JB, N], dt.float32, tag="out")
                nc.scalar.copy(out_sb, acc)
                nc.sync.dma_start(out=dram[pfx + "_out"], in_=out_sb)
                if tc_sb is not None:
                    nc.sync.dma_start(out=dram[pfx + "_tc"], in_=tc_sb)

    nc.finalize()
    return nc


def _get_nc(repeat=1):
    key = ("nc", repeat)
    if key not in _CACHE:
        _CACHE[key] = _build_nc(repeat=repeat)
    return _CACHE[key]


def _prep_inputs(teacher, student):
    in_maps = []
    prepped = {}
    m05i = (-0.5 * np.eye(128)).astype(ml_dtypes.bfloat16)
    for pfx, x in (("t", teacher), ("s", student)):
        xb = np.asarray(x, np.float32).astype(ml_dtypes.bfloat16)   # [N, D] bf16
        xtb = np.ascontiguousarray(xb.T)                            # [D, N] bf16
        xtb32 = xtb.astype(np.float32)  # bf16-rounded values, exact in fp32
        prepped[pfx] = (xb, xtb, xtb32)
    for k in range(NCORES):
        m = {"m05i": m05i}
        for pfx in ("t", "s"):
            xb, xtb, xtb32 = prepped[pfx]
            m[pfx + "_xt"] = xtb
            m[pfx + "_xr"] = xb
            m[pfx + "_xjr"] = np.ascontiguousarray(xb[k::8, :])
            m[pfx + "_xj"] = np.ascontiguousarray(xtb[:, k::8])
            m[pfx + "_jp1"] = np.ascontiguousarray(xtb32[:, k::8] + 1.0)
            m[pfx + "_jm1"] = np.ascontiguousarray(xtb32[:, k::8] - 1.0)
        in_maps.append(m)
    return in_maps


def _assemble(blocks):
    """blocks: list of [JB, N] per core; returns the full symmetric [N, N]."""
    U = np.zeros((N, N), np.float64)
    for k in range(NCORES):
        b = blocks[k].astype(np.float64)
        for jl in range(JB):
            j = 8 * jl + k
            U[j, j + 1:] = b[jl, j + 1:]
    return U + U.T


def run_device(teacher, student, **kwargs):
    """Run the device part; returns (T, S) full pair-sum matrices and results."""
    from concourse.bass_utils import run_bass_kernel_spmd

    nc = _get_nc()
    in_maps = _prep_inputs(teacher, student)
    res = run_bass_kernel_spmd(nc, in_maps, core_ids=list(range(NCORES)), **kwargs)
    T = _assemble([res.results[k]["t_out"] for k in range(NCORES)])
    S = _assemble([res.results[k]["s_out"] for k in range(NCORES)])
    return T, S, res


def kernel(teacher, student):
    teacher = np.asarray(teacher)
    student = np.asarray(student)
    T, S, _ = run_device(teacher, student)
    out = np.abs(T / T.mean() - S / S.mean()).sum()
    return np.float32(out)


if __name__ == "__main__":
    rng = np.random.default_rng(0)
    t = rng.standard_normal((N, D)).astype(np.float32)
    s = rng.standard_normal((N, D)).astype(np.float32)
    print(kernel(t, s))



# revision 5
# speedup vs baseline: 1.2456x; 1.2456x over previous
"""Trainium2 Bass kernel for nn_DistanceLoss (pairwise SmoothL1 distance loss).

reference:
    t[i,j] = sum_d smoothl1(x[i,d] - x[j,d])   (beta=1)  for x in {teacher, student}
    loss = sum |t/mean(t) - s/mean(s)|

identity used (per pair, with d = x_i - x_j):
    smoothl1(d) = 0.5 d^2 - 0.5 relu(|d|-1)^2
    sum_d 0.5 d^2 = 0.5 n_i + 0.5 n_j - G_ij       (Gram decomposition)

The device computes, per core, rows j == k (mod 8) of (-G_ij + corr_ij) for
the upper triangle (corr = -0.5 sum_d relu(|d|-1)^2, accumulated into PSUM by
indicator matmuls). The rank-1 outer-sum 0.5 n_i + 0.5 n_j and the final
mean-normalize/abs-diff reduction are O(N^2) host work in float64.

The nonlinear correction c2 = relu(|x_i - x_j| - 1)^2 is split across engines:
  - fused path (most j's): a custom DVE op (ABSOLUTE_DIFF/sub/max/mul, 4 ALU
    slices) with a hand-written 2X_1PORT uop program (two bf16 elements per
    cycle) and a 2-state subdim FSM so one instruction covers two partition
    tiles (scalar xj switches C0 -> C1 at the subdim boundary).
  - ACT path (largest-fd j's): Abs on the Scalar engine (bias = xj, scale=-1),
    relu(.-1) via tensor_scalar (DVE 4x or Pool), Square via tensor_tensor
    (ACT / DVE 2x / Pool).
  - Pool path (smallest-fd j's): 3-pass tensor_scalar/tensor_tensor chain.
"""

import os
import sys

for _p in ("/opt/trn_rl_repo", "/root/.axon_site/_ro/trn_rl_repo"):
    if _p not in sys.path:
        sys.path.insert(0, _p)

import numpy as np
import ml_dtypes

N = 512
D = 512
NCORES = 8
JB = N // NCORES  # 64 rows of the pair matrix per core
NT = D // 128  # 4 partition tiles of the transposed layout

# work split (see calc in transcript): jl < K -> ACT path; jl >= JB-NPOOL ->
# Pool path; rest -> fused DVE path. Within the ACT path, Square runs on ACT
# for jl < NSQA, on Pool for jl >= K-NSQP, else on DVE; the relu step runs on
# Pool for jl >= K-VP, else on DVE.
K = int(os.environ.get("SL1_K", "9"))
NSQA = int(os.environ.get("SL1_NSQA", "6"))
NSQP = int(os.environ.get("SL1_NSQP", "3"))
NPOOL = int(os.environ.get("SL1_NPOOL", "8"))
VP = int(os.environ.get("SL1_VP", "4"))
PERF = os.environ.get("SL1_PERF", "1") == "1"  # declare 2X_1PORT on custom ops
SUBD = os.environ.get("SL1_SUBD", "1") == "1"  # 2-subdim paired custom op
RUNWAY = int(os.environ.get("SL1_RUN", "2"))
COPY_ENG = os.environ.get("SL1_COPY", "act")

_CACHE = {}


def _fd(jl):
    return N - 8 * jl


def _register_custom_ops():
    """Register the smooth-l1 correction ops:
      SL1P_ANT: paired 2-subdim op, out[:,r,:] = relu(|in0[:,r,:] - s_r| - 1)^2
      SL1X_ANT: plain single-tile op,  out = relu(|in0 - s0| - 1)^2
    Both carry a REGULAR program (from the spec compiler) and a hand-written
    2X_1PORT program (4 ALU slices per element, lo in blocks 0-3, hi in 4-7),
    registered via the perf-mode table slots."""
    import copy as _copy

    import concourse.dve_ops as dve_ops
    from concourse.dve_spec import Spec, Src0, C0, Zero, One, maxx, sq, lower, Bin
    from concourse.dve_uop import (
        DveOpSpec,
        UopConfig,
        UopDpConfig,
        AluOp,
        AluInp,
        InpSel,
        OutSel,
        OutPath,
        DelayInp,
        Trigger,
        ENABLE,
    )

    existing = {op.name: op for op in dve_ops.OPS}
    if "SL1P_ANT" in existing and "SL1X_ANT" in existing:
        return existing["SL1P_ANT"], existing["SL1X_ANT"]

    body = sq(maxx(Bin(AluOp.ABSOLUTE_DIFF, Src0, C0) - One, Zero))

    def _ref_plain(in0, in1, s0, s1, imm2):
        x = in0.astype(np.float32)
        return np.square(np.maximum(np.abs(x - s0) - 1.0, 0.0)).astype(np.float32)

    def _ref_pair(in0, in1, s0, s1, imm2):
        x = in0.astype(np.float32)
        out = np.empty_like(x)
        out[:, 0] = np.square(np.maximum(np.abs(x[:, 0] - s0) - 1.0, 0.0))
        out[:, 1] = np.square(np.maximum(np.abs(x[:, 1] - s1) - 1.0, 0.0))
        return out

    base = lower(Spec(body=body, reference=_ref_plain), ver="v3")[0]

    ST, SD, NONE = (
        Trigger.SRC_TENSOR_DONE,
        Trigger.SUB_DIM_DONE,
        Trigger.NONE,
    )

    def _patch(u, const_sel, trig, nxt):
        v = _copy.deepcopy(u)
        for i in range(len(v.inp)):
            if v.inp_enable[i] and v.inp[i] == InpSel.CONST_0:
                v.inp[i] = const_sel
        v.trigger = trig
        v.next_uop = nxt
        return v

    def _mk2x(const_sel, trig, nxt):
        u = UopConfig()
        u.enable_input(InpSel.SRC_0, 1)
        u.enable_input(const_sel, 2)
        u.enable_input(InpSel.ONE_F32, 3)
        u.enable_input(InpSel.ZERO, 4)
        u.enable_input(InpSel.SRC_0_HI, 5)
        P = AluInp.PREV_ALU_OUT
        Dl = (
            AluInp.PREV_DELAY_0,
            AluInp.PREV_DELAY_1,
            AluInp.PREV_DELAY_2,
            AluInp.PREV_DELAY_3,
            AluInp.PREV_DELAY_4,
        )
        dp = u.datapath_config
        # lo element: blocks 0-3; chains: 0=src_lo 1=const 2=one 3=zero 4=src_hi
        dp[0] = (
            UopDpConfig()
            .enable_alu(AluOp.ABSOLUTE_DIFF, Dl[0], Dl[1])
            .pass_through_delay(1, 2, 3, 4)
        )
        dp[1] = (
            UopDpConfig()
            .enable_alu(AluOp.SUBTRACT, P, Dl[2])
            .pass_through_delay(1, 2, 3, 4)
        )
        dp[2] = (
            UopDpConfig()
            .enable_alu(AluOp.MAX, P, Dl[3])
            .pass_through_delay(1, 2, 3, 4)
        )
        dp[3] = (
            UopDpConfig()
            .enable_alu(AluOp.MULTIPLY, P, P)
            .pass_through_delay(1, 2, 3, 4)
        )
        # hi element: blocks 4-7; lo result rides chain 0 from block 4 on
        dp[4] = (
            UopDpConfig()
            .enable_alu(AluOp.ABSOLUTE_DIFF, Dl[4], Dl[1])
            .enable_delay_from_src(DelayInp.PREV_ALU_OUT, 0)
            .pass_through_delay(2, 3)
        )
        dp[5] = (
            UopDpConfig()
            .enable_alu(AluOp.SUBTRACT, P, Dl[2])
            .pass_through_delay(0, 3)
        )
        dp[6] = UopDpConfig().enable_alu(AluOp.MAX, P, Dl[3]).pass_through_delay(0)
        dp[7] = UopDpConfig().enable_alu(AluOp.MULTIPLY, P, P).pass_through_delay(0)
        u.enable_output(OutSel.DELAY_0, OutPath.WR0_LO)
        u.enable_output(OutSel.ALU_OUT, OutPath.WR0_HI)
        u.require_inp0 = ENABLE
        u.trigger = trig
        u.next_uop = nxt
        return u

    def _reg(name, spec, regular, uops_2x, subdim):
        row = dve_ops._CUSTOM_DVE_ROW_BASE + len(dve_ops.OPS)
        dspec = DveOpSpec(
            name=name,
            opcode=row,
            uops=regular,
            uops_2x=uops_2x,
            perf_max=1,
            rd1_en=False,
        )
        for u in regular + uops_2x:
            u.validate("v3")
        op = dve_ops.DveOp(
            name, spec, subdim=subdim, uops_sha={"v3": dspec.sha("v3")}
        )
        dve_ops.OPS.append(op)
        dve_ops._SUB_OPCODE_FOR_NAME[name] = row
        dve_ops.CUSTOM_DVE_SPECS[name] = spec
        dve_ops._COMPILE_CACHE[(name, "v3")] = dspec
        return op

    sl1p = _reg(
        "SL1P_ANT",
        Spec(body=body, reference=_ref_pair),
        [
            _patch(base, InpSel.CONST_0, (ST, SD, NONE), (0, 1, 0)),
            _patch(base, InpSel.CONST_1, (ST, SD, NONE), (0, 1, 0)),
        ],
        [
            _mk2x(InpSel.CONST_0, (ST, SD, NONE), (0, 1, 0)),
            _mk2x(InpSel.CONST_1, (ST, SD, NONE), (0, 1, 0)),
        ],
        subdim=True,
    )
    sl1x = _reg(
        "SL1X_ANT",
        Spec(body=body, reference=_ref_plain),
        [_patch(base, InpSel.CONST_0, (ST, NONE, NONE), (0, 0, 0))],
        [_mk2x(InpSel.CONST_0, (ST, NONE, NONE), (0, 0, 0))],
        subdim=False,
    )
    return sl1p, sl1x


def _path(jl):
    if jl < K:
        return "A"
    if jl >= JB - NPOOL:
        return "P"
    return "F"


def _build_nc(repeat=1):
    import concourse.bacc as bacc
    import concourse.tile as tile
    from concourse import mybir

    sl1p, sl1x = _register_custom_ops()

    dt = mybir.dt
    nc = bacc.Bacc("TRN2", target_bir_lowering=False, debug=False,
                   num_devices=NCORES)

    dram = {}
    for pfx in ("t", "s"):
        dram[pfx + "_xt"] = nc.dram_tensor(pfx + "_xt", [D, N], dt.bfloat16,
                                           kind="ExternalInput").ap()
        dram[pfx + "_xj32"] = nc.dram_tensor(pfx + "_xj32", [D, JB], dt.float32,
                                             kind="ExternalInput").ap()
        dram[pfx + "_out"] = nc.dram_tensor(pfx + "_out", [JB, N], dt.float32,
                                            kind="ExternalOutput").ap()

    with tile.TileContext(nc) as tc:
        import contextlib

        with contextlib.ExitStack() as ctx:
            singles = ctx.enter_context(tc.tile_pool(name="singles", bufs=1))
            qpool = ctx.enter_context(tc.tile_pool(name="qpool", bufs=8))
            apool = ctx.enter_context(tc.tile_pool(name="apool", bufs=4))
            vpool = ctx.enter_context(tc.tile_pool(name="vpool", bufs=4))
            ppool = ctx.enter_context(tc.tile_pool(name="ppool", bufs=3))
            opool = ctx.enter_context(tc.tile_pool(name="opool", bufs=2))
            psp = ctx.enter_context(tc.tile_pool(name="psp", bufs=2, space="PSUM"))

            # sliding -0.5 indicator for the correction reduction matmuls
            zo = singles.tile([128, 128], dt.bfloat16)
            nc.gpsimd.memset(zo, 0.0)
            nc.gpsimd.memset(zo[:, 63:64], -0.5)

            _ord = ("s", "t") if os.environ.get("SL1_SWAP", "") == "1" else ("t", "s")
            _phases = [p for _ in range(repeat) for p in _ord]

            # all input DMAs upfront so the SP DMA queue never blocks a
            # later phase's loads behind an earlier phase's output
            xt_all, xj32_all, negxj_all = {}, {}, {}
            for pfx in _phases:
                # xt pairs: [128, 2, N] tiles so one paired DVE op spans two
                # partition tiles (subrow r holds d-rows 128*(2u+r)+p)
                xt = []
                for u in range(2):
                    x = singles.tile([128, 2, N], dt.bfloat16, tag=f"{pfx}_xt{u}")
                    for r in range(2):
                        t = 2 * u + r
                        nc.sync.dma_start(
                            out=x[:, r, :],
                            in_=dram[pfx + "_xt"][128 * t:128 * (t + 1), :])
                    xt.append(x)
                xj32 = []
                for t in range(NT):
                    p = singles.tile([128, JB], dt.float32, tag=f"{pfx}_xj32{t}")
                    nc.sync.dma_start(
                        out=p, in_=dram[pfx + "_xj32"][128 * t:128 * (t + 1), :])
                    xj32.append(p)
                # -xj in bf16: stationary for the -G matmuls
                negxj = []
                for t in range(NT):
                    nb = singles.tile([128, JB], dt.bfloat16, tag=f"{pfx}_negxj{t}")
                    nc.gpsimd.tensor_scalar(nb, xj32[t], -1.0, None,
                                            mybir.AluOpType.mult)
                    negxj.append(nb)
                xt_all[pfx], xj32_all[pfx], negxj_all[pfx] = xt, xj32, negxj

            accs, outs = {}, {}
            for _pi, pfx in enumerate(_phases):
                xt, xj32, negxj = xt_all[pfx], xj32_all[pfx], negxj_all[pfx]

                def _xts(t, sl=slice(None)):
                    return xt[t // 2][:, t % 2, sl]

                acc = psp.tile([JB, N], dt.float32, tag=f"{pfx}_acc")
                accs[pfx] = acc
                for t in range(NT):
                    nc.tensor.matmul(acc, negxj[t], _xts(t), start=(t == 0),
                                     stop=False)

                # interleave the three paths so every engine has runnable work
                a_js = [j for j in range(JB) if _path(j) == "A"]
                f_js = [j for j in range(JB) if _path(j) == "F"]
                p_js = [j for j in range(JB) if _path(j) == "P"]
                order = []
                ia = if_ = ip = 0
                for _ in range(min(RUNWAY, len(f_js))):
                    order.append(f_js[if_]); if_ += 1
                INF = float("inf")
                while len(order) < JB:
                    ra = ia / len(a_js) if ia < len(a_js) else INF
                    rf = if_ / len(f_js) if if_ < len(f_js) else INF
                    rp = (ip / len(p_js)) * 1.35 if ip < len(p_js) else INF
                    m = min(ra, rf, rp)
                    if ra == m:
                        order.append(a_js[ia]); ia += 1
                    elif rf == m:
                        order.append(f_js[if_]); if_ += 1
                    else:
                        order.append(p_js[ip]); ip += 1

                n_mm = 0
                total_mm = 4 * JB
                for jl in order:
                    fd = _fd(jl)
                    i0 = N - fd
                    path = _path(jl)
                    q4 = qpool.tile([128, NT, N], dt.bfloat16, tag="q4")
                    if path == "F":
                        if SUBD:
                            for u in range(2):
                                bop = nc.vector._custom_dve(
                                    sl1p,
                                    out=q4[:, 2 * u:2 * u + 2, 0:fd],
                                    in0=xt[u][:, :, i0:N],
                                    s0=xj32[2 * u][:, jl:jl + 1],
                                    s1=xj32[2 * u + 1][:, jl:jl + 1])
                                if PERF:
                                    bop.ins.perf_max = 1
                        else:
                            for t in range(NT):
                                bop = nc.vector._custom_dve(
                                    sl1x,
                                    out=q4[:, t, 0:fd],
                                    in0=_xts(t, slice(i0, N)),
                                    s0=xj32[t][:, jl:jl + 1])
                                if PERF:
                                    bop.ins.perf_max = 1
                    elif path == "A":
                        a4 = apool.tile([128, NT, N], dt.bfloat16, tag="a4")
                        for t in range(NT):
                            nc.scalar.activation(a4[:, t, 0:fd],
                                                 _xts(t, slice(i0, N)),
                                                 mybir.ActivationFunctionType.Abs,
                                                 bias=xj32[t][:, jl:jl + 1],
                                                 scale=-1.0)
                        v4 = vpool.tile([128, NT, N], dt.bfloat16, tag="v4")
                        veng = nc.gpsimd if jl >= K - VP else nc.vector
                        veng.tensor_scalar(v4[:, :, 0:fd], a4[:, :, 0:fd],
                                           1.0, 0.0, mybir.AluOpType.subtract,
                                           mybir.AluOpType.max)
                        if jl < NSQA:
                            nc.scalar.activation(
                                q4[:, :, 0:fd], v4[:, :, 0:fd],
                                mybir.ActivationFunctionType.Square,
                                bias=0.0, scale=1.0)
                        else:
                            sqeng = nc.gpsimd if jl >= K - NSQP else nc.vector
                            sqeng.tensor_tensor(q4[:, :, 0:fd], v4[:, :, 0:fd],
                                                v4[:, :, 0:fd],
                                                mybir.AluOpType.mult)
                    else:  # "P"
                        p4 = ppool.tile([128, NT, N], dt.bfloat16, tag="p4")
                        for t in range(NT):
                            nc.gpsimd.tensor_scalar(p4[:, t, 0:fd],
                                                    _xts(t, slice(i0, N)),
                                                    xj32[t][:, jl:jl + 1], 0.0,
                                                    mybir.AluOpType.subtract,
                                                    mybir.AluOpType.abs_max)
                        v4 = vpool.tile([128, NT, N], dt.bfloat16, tag="v4")
                        nc.gpsimd.tensor_scalar(v4[:, :, 0:fd], p4[:, :, 0:fd],
                                                1.0, 0.0,
                                                mybir.AluOpType.subtract,
                                                mybir.AluOpType.max)
                        nc.gpsimd.tensor_tensor(q4[:, :, 0:fd], v4[:, :, 0:fd],
                                                v4[:, :, 0:fd],
                                                mybir.AluOpType.mult)
                    for t in range(NT):
                        n_mm += 1
                        nc.tensor.matmul(acc[:, i0:N], zo[:, 63 - jl:127 - jl],
                                         q4[:, t, 0:fd],
                                         start=False, stop=(n_mm == total_mm))

            # copies + output DMAs at the very end: nothing queues behind them
            for pfx in _phases:
                out_sb = opool.tile([JB, N], dt.float32, tag=f"{pfx}_out")
                if COPY_ENG == "pool":
                    nc.gpsimd.tensor_copy(out_sb, accs[pfx])
                elif COPY_ENG == "dve":
                    nc.vector.tensor_copy(out_sb, accs[pfx])
                else:
                    nc.scalar.copy(out_sb, accs[pfx])
                nc.sync.dma_start(out=dram[pfx + "_out"], in_=out_sb)

    nc.finalize()
    return nc


def _get_nc(repeat=1):
    key = ("nc", repeat)
    if key not in _CACHE:
        _CACHE[key] = _build_nc(repeat=repeat)
    return _CACHE[key]


def _prep_inputs(teacher, student):
    in_maps = []
    prepped = {}
    for pfx, x in (("t", teacher), ("s", student)):
        xb = np.asarray(x, np.float32).astype(ml_dtypes.bfloat16)   # [N, D] bf16
        xtb = np.ascontiguousarray(xb.T)                            # [D, N] bf16
        xtb32 = xtb.astype(np.float32)  # bf16-rounded values, exact in fp32
        prepped[pfx] = (xtb, xtb32)
    for k in range(NCORES):
        m = {}
        for pfx in ("t", "s"):
            xtb, xtb32 = prepped[pfx]
            m[pfx + "_xt"] = xtb
            m[pfx + "_xj32"] = np.ascontiguousarray(xtb32[:, k::8])
        in_maps.append(m)
    return in_maps


def _assemble(blocks, n):
    """blocks: list of [JB, N] device rows (-G + corr) per core; n: [N] fp64
    squared-norm vector. Returns the full symmetric pair-sum matrix [N, N]."""
    U = np.zeros((N, N), np.float64)
    for k in range(NCORES):
        b = blocks[k].astype(np.float64)
        for jl in range(JB):
            j = 8 * jl + k
            U[j, j + 1:] = b[jl, j + 1:]
    T = U + U.T
    M = 0.5 * (n[:, None] + n[None, :])
    np.fill_diagonal(M, 0.0)
    T += M
    np.fill_diagonal(T, 0.0)
    return T


def run_device(teacher, student, **kwargs):
    """Run the device part; returns (T, S) full pair-sum matrices and results."""
    from concourse.bass_utils import run_bass_kernel_spmd

    nc = _get_nc()
    in_maps = _prep_inputs(teacher, student)
    res = run_bass_kernel_spmd(nc, in_maps, core_ids=list(range(NCORES)), **kwargs)
    ns = {}
    for pfx, x in (("t", teacher), ("s", student)):
        xb32 = np.asarray(x, np.float32).astype(ml_dtypes.bfloat16).astype(np.float64)
        ns[pfx] = np.square(xb32).sum(axis=1)
    T = _assemble([res.results[k]["t_out"] for k in range(NCORES)], ns["t"])
    S = _assemble([res.results[k]["s_out"] for k in range(NCORES)], ns["s"])
    return T, S, res


def kernel(teacher, student):
    teacher = np.asarray(teacher)
    student = np.asarray(student)
    T, S, _ = run_device(teacher, student)
    out = np.abs(T / T.mean() - S / S.mean()).sum()
    return np.float32(out)


if __name__ == "__main__":
    rng = np.random.default_rng(0)
    t = rng.standard_normal((N, D)).astype(np.float32)
    s = rng.standard_normal((N, D)).astype(np.float32)
    print(kernel(t, s))
